# revision 9
# baseline (speedup 1.0000x reference)
"""AcidBaseDense Trainium2 kernel.

Math (reference, f32):
    bw   = sign(clip(w, -1, 1))                    in {-1, 0, +1}
    h    = 10^(-x);  oh = 1e-14 / h                (oh <= 1e-13 << f32 eps of h)
    r    = (h*0.1) @ bw - (oh*0.1) @ bw            == (h*0.1) @ bw  at f32 precision
    conc = |r| / 409.6
    ph   = -log10(conc)               if r >= 0
         = -log10(1e-14 / conc)       if r <  0

Kernel strategy:
  * host: pre-transpose x so the device loads x^T with n_in on partitions
    (contraction dim must sit on partitions for the PE); binarize the static
    weights (sign(clip(w)) is deterministic weight preprocessing, exact in
    fp16/fp8); shard 2-way over batch x 4-way over n_out across 8 cores.
  * device: A^T = 0.1*10^(-x^T) via one ACT Exp pass.  Precision split:
      A = A_hi (fp16) + A_lo,  |A_lo| <= 2^-11 |A|
    hi pass: fp16 matmul (full rate), S in fp16 (exact +-1/0)
    lo pass: fp8e4 DoubleRow matmul (0.5 cyc/row) on A_lo * 2^18 quantized
      to e4m3 (error 2^-4 * 2^-11 = 2^-15 per term ~ f32-grade), S in fp8.
    Accumulate each pass in its own PSUM bank; combine in the epilogue:
      r = psum_hi + 2^-18 * psum_lo.
  * epilogue in log space avoids any division:
      L = ln(max(|r|, tiny));  u = L*(-1/ln10) + (log10(409.6) - 7)
      ph = 7 + sign(r) * u
"""

import os
import sys

for _p in ("/opt/trn_rl_repo", "/root/.axon_site/_ro/trn_rl_repo"):
    if os.path.isdir(_p) and _p not in sys.path:
        sys.path.insert(0, _p)

import numpy as np

BATCH = 4096
N_IN = 4096
N_OUT = 4096
B_GROUPS = 2           # batch shards
N_GROUPS = 4           # n_out shards
B_SH = BATCH // B_GROUPS      # 2048 batch rows per core
N_SH = N_OUT // N_GROUPS      # 1024 out cols per core
KT = N_IN // 128              # 32 contraction tiles
MT = B_SH // 128              # 16 batch tiles per core
NCHUNK = 2                    # two 512-wide PSUM chunks per batch tile

LN10 = float(np.log(10.0))
U_SCALE = -1.0 / LN10
U_BIAS = float(np.log10(4096 * 0.1) - 7.0)
LO_SCALE = float(2.0 ** 18)
LO_INV = float(2.0 ** -18)

_CACHED = {}


def _build_nc():
    import concourse.bacc as bacc
    import concourse.mybir as mybir
    import concourse.tile as tile

    F32 = mybir.dt.float32
    FP16 = mybir.dt.float16
    FP8 = mybir.dt.float8e4
    BF16 = mybir.dt.bfloat16
    AFT = mybir.ActivationFunctionType

    nc = bacc.Bacc(trn_type="TRN2")
    xt_d = nc.dram_tensor("xt", [N_IN, B_SH], F32, kind="ExternalInput")
    s16_d = nc.dram_tensor("s16", [N_IN, N_SH], FP16, kind="ExternalInput")
    s8_d = nc.dram_tensor("s8", [N_IN, N_SH], FP8, kind="ExternalInput")
    y_d = nc.dram_tensor("y", [B_SH, N_SH], F32, kind="ExternalOutput")

    with tile.TileContext(nc) as tc:
        with (
            tc.tile_pool(name="spool", bufs=1) as spool,
            tc.tile_pool(name="mpool", bufs=3) as mpool,
            tc.tile_pool(name="epool", bufs=2) as epool,
            tc.tile_pool(name="cpool", bufs=1) as cpool,
            tc.tile_pool(name="ph_pool", bufs=4, space="PSUM") as ph_pool,
            tc.tile_pool(name="pl_pool", bufs=2, space="PSUM") as pl_pool,
        ):
            bias_ln10 = cpool.tile([128, 1], F32, tag="bias")
            nc.gpsimd.memset(bias_ln10[:], -LN10)

            # m-tile prep: load x^T in 8-ktile quarters (separate tiles for
            # fine-grained deps), exp in place, split hi/lo per quarter
            Q = N_IN // 4

            def prep(m):
                src = xt_d[:, m * 128:(m + 1) * 128].rearrange(
                    "(t p) b -> p t b", p=128
                )
                a_hi, a_lo8 = [], []
                for q in range(4):
                    a32q = mpool.tile([128, Q], F32, tag=f"a32q{q}")
                    nc.sync.dma_start(
                        a32q[:].rearrange("p (t b) -> p t b", b=128),
                        src[:, q * (KT // 4):(q + 1) * (KT // 4), :],
                    )
                    nc.scalar.activation(
                        a32q[:], a32q[:], AFT.Exp,
                        bias=bias_ln10[:], scale=-LN10,
                    )
                    # a_hi holds A * 2^18 (exact power-of-2 scale) in fp16;
                    # lo8 = e4m3(A*2^18 - a_hi_scaled) in one fused op
                    ahq = mpool.tile([128, Q], FP16, tag=f"ahiq{q}")
                    nc.vector.tensor_scalar_mul(ahq[:], a32q[:], LO_SCALE)
                    al8q = mpool.tile([128, Q], FP8, tag=f"alo8q{q}")
                    nc.vector.scalar_tensor_tensor(
                        al8q[:], a32q[:], LO_SCALE, ahq[:],
                        op0=mybir.AluOpType.mult,
                        op1=mybir.AluOpType.subtract,
                    )
                    a_hi.append(ahq)
                    a_lo8.append(al8q)
                return a_hi, a_lo8

            # S: host-binarized sign(w), exact in fp16 / fp8
            s16 = spool.tile([128, KT, N_SH], FP16, tag="s16")
            s8 = spool.tile([128, KT, N_SH], FP8, tag="s8")

            def load_s(q):
                tq = KT // 4
                nc.sync.dma_start(
                    s16[:, q * tq:(q + 1) * tq, :],
                    s16_d[q * tq * 128:(q + 1) * tq * 128, :].rearrange(
                        "(t p) n -> p t n", p=128
                    ),
                )
                nc.sync.dma_start(
                    s8[:, q * tq:(q + 1) * tq, :],
                    s8_d[q * tq * 128:(q + 1) * tq * 128, :].rearrange(
                        "(t p) n -> p t n", p=128
                    ),
                )

            def hilo_chain(m, a_hi, a_lo8, n):
                # interleave hi (fp16) and lo (fp8 DoubleRow) matmuls so the
                # DR LDWEIGHTS (~213ns, FWL off) hides under hi streaming
                pt = ph_pool.tile([128, 512], F32, tag="ph")
                pl = pl_pool.tile([128, 512], F32, tag="pl")
                T = KT // 2
                for t in range(T):
                    for j in (0, 1):
                        kt = 2 * t + j
                        q, kq = divmod(kt, KT // 4)
                        nc.tensor.matmul(
                            pt[:],
                            a_hi[q][:, kq * 128:(kq + 1) * 128],
                            s16[:, kt, n * 512:(n + 1) * 512],
                            start=(kt == 0),
                            stop=(kt == KT - 1),
                            skip_group_check=True,
                        )
                    q, tq = divmod(2 * t, KT // 4)
                    lhs = a_lo8[q][:, tq * 128:(tq + 2) * 128].rearrange(
                        "p (j m) -> p j m", j=2
                    )
                    rhs = s8[:, 2 * t:2 * t + 2, n * 512:(n + 1) * 512]
                    nc.tensor.matmul(
                        pl[:], lhs, rhs,
                        start=(t == 0), stop=(t == T - 1),
                        perf_mode=mybir.MatmulPerfMode.DoubleRow,
                        skip_group_check=True,
                    )
                return pt, pl

            def epilogue(n, y_sb, pt, pl):
                    # r = 2^-18 * (psum_hi + psum_lo), both scaled by 2^18
                    tr = epool.tile([128, 512], F32, tag="tr")
                    nc.scalar.activation(
                        tr[:], pl[:], AFT.Copy, bias=0.0, scale=LO_INV
                    )
                    nc.vector.scalar_tensor_tensor(
                        tr[:], pt[:], LO_INV, tr[:],
                        op0=mybir.AluOpType.mult,
                        op1=mybir.AluOpType.add,
                    )
                    # ph = 7 + sign(r) * (ln|r| * U_SCALE + U_BIAS)
                    tab = epool.tile([128, 512], F32, tag="tab")
                    nc.scalar.activation(tab[:], tr[:], AFT.Abs)
                    nc.vector.tensor_scalar_max(tab[:], tab[:], 1e-30)
                    tln = epool.tile([128, 512], F32, tag="tln")
                    nc.scalar.activation(tln[:], tab[:], AFT.Ln)
                    nc.scalar.activation(
                        tln[:], tln[:], AFT.Copy, bias=U_BIAS, scale=U_SCALE
                    )
                    tsg = epool.tile([128, 512], F32, tag="tsg")
                    nc.scalar.activation(tsg[:], tr[:], AFT.Sign)
                    ych = y_sb[:, n * 512:(n + 1) * 512]
                    nc.vector.tensor_mul(ych, tsg[:], tln[:])
                    nc.vector.tensor_scalar_add(ych, ych, 7.0)

            # ---- pipeline: prep one m-tile ahead
            load_s(0)
            a_prev = prep(0)
            for q in range(1, 4):
                load_s(q)
            for m in range(MT):
                a_hi, a_lo8 = a_prev
                y_sb = epool.tile([128, N_SH], F32, tag="y_sb")
                for n in range(NCHUNK):
                    pt, pl = hilo_chain(m, a_hi, a_lo8, n)
                    if n == 0 and m + 1 < MT:
                        a_prev = prep(m + 1)
                    epilogue(n, y_sb, pt, pl)
                    nc.gpsimd.dma_start(
                        y_d[m * 128:(m + 1) * 128, n * 512:(n + 1) * 512],
                        y_sb[:, n * 512:(n + 1) * 512],
                    )

    nc.compile()
    return nc


def kernel(x: np.ndarray, w: np.ndarray) -> np.ndarray:
    import ml_dtypes
    from concourse.bass_utils import run_bass_kernel_spmd

    assert x.shape == (BATCH, N_IN) and w.shape == (N_IN, N_OUT)
    x = np.ascontiguousarray(x, dtype=np.float32)
    w = np.ascontiguousarray(w, dtype=np.float32)

    if "nc" not in _CACHED:
        _CACHED["nc"] = _build_nc()
    nc = _CACHED["nc"]

    # static weight preprocessing: sign(clip(w)), exactly representable
    s = np.sign(np.clip(w, -1.0, 1.0))
    s16_full = s.astype(np.float16)
    s8_full = s.astype(ml_dtypes.float8_e4m3)

    in_maps = []
    for c in range(8):
        bg, ng = divmod(c, N_GROUPS)
        xt_sh = np.ascontiguousarray(x[bg * B_SH:(bg + 1) * B_SH, :].T)
        in_maps.append({
            "xt": xt_sh,
            "s16": np.ascontiguousarray(s16_full[:, ng * N_SH:(ng + 1) * N_SH]),
            "s8": np.ascontiguousarray(s8_full[:, ng * N_SH:(ng + 1) * N_SH]),
        })

    trace = os.environ.get("PH_KERNEL_TRACE", "") == "1"
    res = run_bass_kernel_spmd(
        nc, in_maps, core_ids=list(range(8)), trace=trace,
        **({"trace_cores": list(range(8))} if trace else {}),
    )
    if trace:
        _CACHED["last_result"] = res

    y = np.empty((BATCH, N_OUT), dtype=np.float32)
    for c, r in enumerate(res.results):
        bg, ng = divmod(c, N_GROUPS)
        y[bg * B_SH:(bg + 1) * B_SH, ng * N_SH:(ng + 1) * N_SH] = r["y"]
    return y


# revision 10
# speedup vs baseline: 1.0429x; 1.0429x over previous
"""AcidBaseDense Trainium2 kernel.

Math (reference, f32):
    bw   = sign(clip(w, -1, 1))                    in {-1, 0, +1}
    h    = 10^(-x);  oh = 1e-14 / h                (oh <= 1e-13 << f32 eps of h)
    r    = (h*0.1) @ bw - (oh*0.1) @ bw            == (h*0.1) @ bw  at f32 precision
    conc = |r| / 409.6
    ph   = -log10(conc)               if r >= 0
         = -log10(1e-14 / conc)       if r <  0

Kernel strategy:
  * host: pre-transpose x so the device loads x^T with n_in on partitions
    (contraction dim must sit on partitions for the PE); binarize the static
    weights (sign(clip(w)) is deterministic weight preprocessing, exact in
    fp16/fp8); shard 2-way over batch x 4-way over n_out across 8 cores.
  * device: A^T = 0.1*10^(-x^T) via one ACT Exp pass.  Precision split:
      A = A_hi (fp16) + A_lo,  |A_lo| <= 2^-11 |A|
    hi pass: fp16 matmul (full rate), S in fp16 (exact +-1/0)
    lo pass: fp8e4 DoubleRow matmul (0.5 cyc/row) on A_lo * 2^18 quantized
      to e4m3 (error 2^-4 * 2^-11 = 2^-15 per term ~ f32-grade), S in fp8.
    Accumulate each pass in its own PSUM bank; combine in the epilogue:
      r = psum_hi + 2^-18 * psum_lo.
  * epilogue in log space avoids any division:
      L = ln(max(|r|, tiny));  u = L*(-1/ln10) + (log10(409.6) - 7)
      ph = 7 + sign(r) * u
"""

import os
import sys

for _p in ("/opt/trn_rl_repo", "/root/.axon_site/_ro/trn_rl_repo"):
    if os.path.isdir(_p) and _p not in sys.path:
        sys.path.insert(0, _p)

import numpy as np

BATCH = 4096
N_IN = 4096
N_OUT = 4096
B_GROUPS = 2           # batch shards
N_GROUPS = 4           # n_out shards
B_SH = BATCH // B_GROUPS      # 2048 batch rows per core
N_SH = N_OUT // N_GROUPS      # 1024 out cols per core
KT = N_IN // 128              # 32 contraction tiles
MT = B_SH // 128              # 16 batch tiles per core
NCHUNK = 2                    # two 512-wide PSUM chunks per batch tile

LN10 = float(np.log(10.0))
U_SCALE = -1.0 / LN10
U_BIAS = float(np.log10(4096 * 0.1) - 7.0)
LO_SCALE = float(2.0 ** 18)
LO_INV = float(2.0 ** -18)

_CACHED = {}


def _build_nc():
    import concourse.bacc as bacc
    import concourse.mybir as mybir
    import concourse.tile as tile

    F32 = mybir.dt.float32
    FP16 = mybir.dt.float16
    FP8 = mybir.dt.float8e4
    BF16 = mybir.dt.bfloat16
    AFT = mybir.ActivationFunctionType

    nc = bacc.Bacc(trn_type="TRN2")
    xt_d = nc.dram_tensor("xt", [N_IN, B_SH], F32, kind="ExternalInput")
    s8_d = nc.dram_tensor("s8", [N_IN, N_SH], FP8, kind="ExternalInput")
    y_d = nc.dram_tensor("y", [B_SH, N_SH], F32, kind="ExternalOutput")

    with tile.TileContext(nc) as tc:
        with (
            tc.tile_pool(name="spool", bufs=1) as spool,
            tc.tile_pool(name="mpool", bufs=3) as mpool,
            tc.tile_pool(name="epool", bufs=2) as epool,
            tc.tile_pool(name="cpool", bufs=1) as cpool,
            tc.tile_pool(name="ph_pool", bufs=4, space="PSUM") as ph_pool,
            tc.tile_pool(name="pl_pool", bufs=2, space="PSUM") as pl_pool,
        ):
            bias_ln10 = cpool.tile([128, 1], F32, tag="bias")
            nc.gpsimd.memset(bias_ln10[:], -LN10)

            # m-tile prep: load x^T in 8-ktile quarters (separate tiles for
            # fine-grained deps), exp in place, split hi/lo per quarter
            Q = N_IN // 4

            def prep(m):
                src = xt_d[:, m * 128:(m + 1) * 128].rearrange(
                    "(t p) b -> p t b", p=128
                )
                a_hi, a_lo8 = [], []
                for q in range(4):
                    a32q = mpool.tile([128, Q], F32, tag=f"a32q{q}")
                    nc.sync.dma_start(
                        a32q[:].rearrange("p (t b) -> p t b", b=128),
                        src[:, q * (KT // 4):(q + 1) * (KT // 4), :],
                    )
                    nc.scalar.activation(
                        a32q[:], a32q[:], AFT.Exp,
                        bias=bias_ln10[:], scale=-LN10,
                    )
                    # a_hi holds A * 2^18 (exact power-of-2 scale) in fp16;
                    # lo8 = e4m3(A*2^18 - a_hi_scaled) in one fused op
                    ahq = mpool.tile([128, Q], FP16, tag=f"ahiq{q}")
                    nc.vector.tensor_scalar_mul(ahq[:], a32q[:], LO_SCALE)
                    al8q = mpool.tile([128, Q], FP8, tag=f"alo8q{q}")
                    nc.vector.scalar_tensor_tensor(
                        al8q[:], a32q[:], LO_SCALE, ahq[:],
                        op0=mybir.AluOpType.mult,
                        op1=mybir.AluOpType.subtract,
                    )
                    a_hi.append(ahq)
                    a_lo8.append(al8q)
                return a_hi, a_lo8

            # S: host-binarized sign(w), exact in fp8 (hi pass reads it as
            # the moving operand of a mixed fp16 x fp8 matmul - bit-exact)
            s8 = spool.tile([128, KT, N_SH], FP8, tag="s8")

            def load_s(q):
                tq = KT // 8
                nc.sync.dma_start(
                    s8[:, q * tq:(q + 1) * tq, :],
                    s8_d[q * tq * 128:(q + 1) * tq * 128, :].rearrange(
                        "(t p) n -> p t n", p=128
                    ),
                )

            def hilo_chain(m, a_hi, a_lo8, n):
                # interleave hi (fp16) and lo (fp8 DoubleRow) matmuls so the
                # DR LDWEIGHTS (~213ns, FWL off) hides under hi streaming
                pt = ph_pool.tile([128, 512], F32, tag="ph")
                pl = pl_pool.tile([128, 512], F32, tag="pl")
                T = KT // 2
                for t in range(T):
                    for j in (0, 1):
                        kt = 2 * t + j
                        q, kq = divmod(kt, KT // 4)
                        nc.tensor.matmul(
                            pt[:],
                            a_hi[q][:, kq * 128:(kq + 1) * 128],
                            s8[:, kt, n * 512:(n + 1) * 512],
                            start=(kt == 0),
                            stop=(kt == KT - 1),
                            skip_group_check=True,
                        )
                    q, tq = divmod(2 * t, KT // 4)
                    lhs = a_lo8[q][:, tq * 128:(tq + 2) * 128].rearrange(
                        "p (j m) -> p j m", j=2
                    )
                    rhs = s8[:, 2 * t:2 * t + 2, n * 512:(n + 1) * 512]
                    nc.tensor.matmul(
                        pl[:], lhs, rhs,
                        start=(t == 0), stop=(t == T - 1),
                        perf_mode=mybir.MatmulPerfMode.DoubleRow,
                        skip_group_check=True,
                    )
                return pt, pl

            def epilogue(n, y_sb, pt, pl):
                    # r = 2^-18 * (psum_hi + psum_lo), both scaled by 2^18
                    tr = epool.tile([128, 512], F32, tag="tr")
                    nc.scalar.activation(
                        tr[:], pl[:], AFT.Copy, bias=0.0, scale=LO_INV
                    )
                    nc.vector.scalar_tensor_tensor(
                        tr[:], pt[:], LO_INV, tr[:],
                        op0=mybir.AluOpType.mult,
                        op1=mybir.AluOpType.add,
                    )
                    # ph = 7 + sign(r) * (ln|r| * U_SCALE + U_BIAS)
                    tab = epool.tile([128, 512], F32, tag="tab")
                    nc.scalar.activation(tab[:], tr[:], AFT.Abs)
                    nc.vector.tensor_scalar_max(tab[:], tab[:], 1e-30)
                    tln = epool.tile([128, 512], F32, tag="tln")
                    nc.scalar.activation(tln[:], tab[:], AFT.Ln)
                    nc.scalar.activation(
                        tln[:], tln[:], AFT.Copy, bias=U_BIAS, scale=U_SCALE
                    )
                    tsg = epool.tile([128, 512], F32, tag="tsg")
                    nc.scalar.activation(tsg[:], tr[:], AFT.Sign)
                    ych = y_sb[:, n * 512:(n + 1) * 512]
                    nc.vector.tensor_mul(ych, tsg[:], tln[:])
                    nc.vector.tensor_scalar_add(ych, ych, 7.0)

            # ---- pipeline: prep one m-tile ahead
            load_s(0)
            load_s(1)
            a_prev = prep(0)
            for q in range(2, 8):
                load_s(q)
            for m in range(MT):
                a_hi, a_lo8 = a_prev
                y_sb = epool.tile([128, N_SH], F32, tag="y_sb")
                for n in range(NCHUNK):
                    pt, pl = hilo_chain(m, a_hi, a_lo8, n)
                    if n == 0 and m + 1 < MT:
                        a_prev = prep(m + 1)
                    epilogue(n, y_sb, pt, pl)
                    nc.gpsimd.dma_start(
                        y_d[m * 128:(m + 1) * 128, n * 512:(n + 1) * 512],
                        y_sb[:, n * 512:(n + 1) * 512],
                    )

    nc.compile()
    return nc


def kernel(x: np.ndarray, w: np.ndarray) -> np.ndarray:
    import ml_dtypes
    from concourse.bass_utils import run_bass_kernel_spmd

    assert x.shape == (BATCH, N_IN) and w.shape == (N_IN, N_OUT)
    x = np.ascontiguousarray(x, dtype=np.float32)
    w = np.ascontiguousarray(w, dtype=np.float32)

    if "nc" not in _CACHED:
        _CACHED["nc"] = _build_nc()
    nc = _CACHED["nc"]

    # static weight preprocessing: sign(clip(w)), exactly representable
    s8_full = np.sign(np.clip(w, -1.0, 1.0)).astype(ml_dtypes.float8_e4m3)

    in_maps = []
    for c in range(8):
        bg, ng = divmod(c, N_GROUPS)
        xt_sh = np.ascontiguousarray(x[bg * B_SH:(bg + 1) * B_SH, :].T)
        in_maps.append({
            "xt": xt_sh,
            "s8": np.ascontiguousarray(s8_full[:, ng * N_SH:(ng + 1) * N_SH]),
        })

    trace = os.environ.get("PH_KERNEL_TRACE", "") == "1"
    res = run_bass_kernel_spmd(
        nc, in_maps, core_ids=list(range(8)), trace=trace,
        **({"trace_cores": list(range(8))} if trace else {}),
    )
    if trace:
        _CACHED["last_result"] = res

    y = np.empty((BATCH, N_OUT), dtype=np.float32)
    for c, r in enumerate(res.results):
        bg, ng = divmod(c, N_GROUPS)
        y[bg * B_SH:(bg + 1) * B_SH, ng * N_SH:(ng + 1) * N_SH] = r["y"]
    return y


# revision 11
# speedup vs baseline: 1.0592x; 1.0157x over previous
"""AcidBaseDense Trainium2 kernel.

Math (reference, f32):
    bw   = sign(clip(w, -1, 1))                    in {-1, 0, +1}
    h    = 10^(-x);  oh = 1e-14 / h                (oh <= 1e-13 << f32 eps of h)
    r    = (h*0.1) @ bw - (oh*0.1) @ bw            == (h*0.1) @ bw  at f32 precision
    conc = |r| / 409.6
    ph   = -log10(conc)               if r >= 0
         = -log10(1e-14 / conc)       if r <  0

Kernel strategy:
  * host: pre-transpose x so the device loads x^T with n_in on partitions
    (contraction dim must sit on partitions for the PE); binarize the static
    weights (sign(clip(w)) is deterministic weight preprocessing, exact in
    fp16/fp8); shard 2-way over batch x 4-way over n_out across 8 cores.
  * device: A^T = 0.1*10^(-x^T) via one ACT Exp pass.  Precision split:
      A = A_hi (fp16) + A_lo,  |A_lo| <= 2^-11 |A|
    hi pass: fp16 matmul (full rate), S in fp16 (exact +-1/0)
    lo pass: fp8e4 DoubleRow matmul (0.5 cyc/row) on A_lo * 2^18 quantized
      to e4m3 (error 2^-4 * 2^-11 = 2^-15 per term ~ f32-grade), S in fp8.
    Accumulate each pass in its own PSUM bank; combine in the epilogue:
      r = psum_hi + 2^-18 * psum_lo.
  * epilogue in log space avoids any division:
      L = ln(max(|r|, tiny));  u = L*(-1/ln10) + (log10(409.6) - 7)
      ph = 7 + sign(r) * u
"""

import os
import sys

for _p in ("/opt/trn_rl_repo", "/root/.axon_site/_ro/trn_rl_repo"):
    if os.path.isdir(_p) and _p not in sys.path:
        sys.path.insert(0, _p)

import numpy as np

BATCH = 4096
N_IN = 4096
N_OUT = 4096
B_GROUPS = 2           # batch shards
N_GROUPS = 4           # n_out shards
B_SH = BATCH // B_GROUPS      # 2048 batch rows per core
N_SH = N_OUT // N_GROUPS      # 1024 out cols per core
KT = N_IN // 128              # 32 contraction tiles
MT = B_SH // 128              # 16 batch tiles per core
NCHUNK = 2                    # two 512-wide PSUM chunks per batch tile

LN10 = float(np.log(10.0))
U_SCALE = -1.0 / LN10
U_BIAS = float(np.log10(4096 * 0.1) - 7.0)
LO_SCALE = float(2.0 ** 18)
LO_INV = float(2.0 ** -18)

_CACHED = {}


def _build_nc():
    import concourse.bacc as bacc
    import concourse.mybir as mybir
    import concourse.tile as tile

    F32 = mybir.dt.float32
    FP16 = mybir.dt.float16
    FP8 = mybir.dt.float8e4
    BF16 = mybir.dt.bfloat16
    AFT = mybir.ActivationFunctionType

    nc = bacc.Bacc(trn_type="TRN2")
    xt_d = nc.dram_tensor("xt", [MT, 128, KT, 128], F32, kind="ExternalInput")
    s8_d = nc.dram_tensor("s8", [128, KT, N_SH], FP8, kind="ExternalInput")
    y_d = nc.dram_tensor("y", [B_SH, N_SH], F32, kind="ExternalOutput")

    with tile.TileContext(nc) as tc:
        with (
            tc.tile_pool(name="spool", bufs=1) as spool,
            tc.tile_pool(name="mpool", bufs=3) as mpool,
            tc.tile_pool(name="epool", bufs=2) as epool,
            tc.tile_pool(name="cpool", bufs=1) as cpool,
            tc.tile_pool(name="ph_pool", bufs=4, space="PSUM") as ph_pool,
            tc.tile_pool(name="pl_pool", bufs=2, space="PSUM") as pl_pool,
        ):
            bias_ln10 = cpool.tile([128, 1], F32, tag="bias")
            nc.gpsimd.memset(bias_ln10[:], -LN10)

            # m-tile prep: load x^T in 8-ktile quarters (separate tiles for
            # fine-grained deps), exp in place, split hi/lo per quarter
            Q = N_IN // 4

            def prep(m):
                a_hi, a_lo8 = [], []
                for q in range(4):
                    a32q = mpool.tile([128, Q], F32, tag=f"a32q{q}")
                    nc.sync.dma_start(
                        a32q[:].rearrange("p (t b) -> p t b", b=128),
                        xt_d[m, :, q * (KT // 4):(q + 1) * (KT // 4), :],
                    )
                    nc.scalar.activation(
                        a32q[:], a32q[:], AFT.Exp,
                        bias=bias_ln10[:], scale=-LN10,
                    )
                    # a_hi holds A * 2^18 (exact power-of-2 scale) in fp16;
                    # lo8 = e4m3(A*2^18 - a_hi_scaled) in one fused op
                    ahq = mpool.tile([128, Q], FP16, tag=f"ahiq{q}")
                    nc.vector.tensor_scalar_mul(ahq[:], a32q[:], LO_SCALE)
                    al8q = mpool.tile([128, Q], FP8, tag=f"alo8q{q}")
                    nc.vector.scalar_tensor_tensor(
                        al8q[:], a32q[:], LO_SCALE, ahq[:],
                        op0=mybir.AluOpType.mult,
                        op1=mybir.AluOpType.subtract,
                    )
                    a_hi.append(ahq)
                    a_lo8.append(al8q)
                return a_hi, a_lo8

            # S: host-binarized sign(w), exact in fp8 (hi pass reads it as
            # the moving operand of a mixed fp16 x fp8 matmul - bit-exact)
            s8 = spool.tile([128, KT, N_SH], FP8, tag="s8")

            def load_s(q):
                tq = KT // 8
                nc.sync.dma_start(
                    s8[:, q * tq:(q + 1) * tq, :],
                    s8_d[:, q * tq:(q + 1) * tq, :],
                )

            def hilo_chain(m, a_hi, a_lo8, n):
                # interleave hi (fp16) and lo (fp8 DoubleRow) matmuls so the
                # DR LDWEIGHTS (~213ns, FWL off) hides under hi streaming
                pt = ph_pool.tile([128, 512], F32, tag="ph")
                pl = pl_pool.tile([128, 512], F32, tag="pl")
                T = KT // 2
                for t in range(T):
                    for j in (0, 1):
                        kt = 2 * t + j
                        q, kq = divmod(kt, KT // 4)
                        nc.tensor.matmul(
                            pt[:],
                            a_hi[q][:, kq * 128:(kq + 1) * 128],
                            s8[:, kt, n * 512:(n + 1) * 512],
                            start=(kt == 0),
                            stop=(kt == KT - 1),
                            skip_group_check=True,
                        )
                    q, tq = divmod(2 * t, KT // 4)
                    lhs = a_lo8[q][:, tq * 128:(tq + 2) * 128].rearrange(
                        "p (j m) -> p j m", j=2
                    )
                    rhs = s8[:, 2 * t:2 * t + 2, n * 512:(n + 1) * 512]
                    nc.tensor.matmul(
                        pl[:], lhs, rhs,
                        start=(t == 0), stop=(t == T - 1),
                        perf_mode=mybir.MatmulPerfMode.DoubleRow,
                        skip_group_check=True,
                    )
                return pt, pl

            def epilogue(n, y_sb, pt, pl):
                    # r = 2^-18 * (psum_hi + psum_lo), both scaled by 2^18
                    tr = epool.tile([128, 512], F32, tag="tr")
                    nc.scalar.activation(
                        tr[:], pl[:], AFT.Copy, bias=0.0, scale=LO_INV
                    )
                    nc.vector.scalar_tensor_tensor(
                        tr[:], pt[:], LO_INV, tr[:],
                        op0=mybir.AluOpType.mult,
                        op1=mybir.AluOpType.add,
                    )
                    # ph = 7 + sign(r) * (ln|r| * U_SCALE + U_BIAS)
                    tab = epool.tile([128, 512], F32, tag="tab")
                    nc.scalar.activation(tab[:], tr[:], AFT.Abs)
                    nc.vector.tensor_scalar_max(tab[:], tab[:], 1e-30)
                    tln = epool.tile([128, 512], F32, tag="tln")
                    nc.scalar.activation(tln[:], tab[:], AFT.Ln)
                    nc.scalar.activation(
                        tln[:], tln[:], AFT.Copy, bias=U_BIAS, scale=U_SCALE
                    )
                    tsg = epool.tile([128, 512], F32, tag="tsg")
                    nc.scalar.activation(tsg[:], tr[:], AFT.Sign)
                    ych = y_sb[:, n * 512:(n + 1) * 512]
                    nc.vector.tensor_mul(ych, tsg[:], tln[:])
                    nc.vector.tensor_scalar_add(ych, ych, 7.0)

            # ---- pipeline: prep one m-tile ahead
            load_s(0)
            load_s(1)
            a_prev = prep(0)
            for q in range(2, 8):
                load_s(q)
            for m in range(MT):
                a_hi, a_lo8 = a_prev
                y_sb = epool.tile([128, N_SH], F32, tag="y_sb")
                for n in range(NCHUNK):
                    pt, pl = hilo_chain(m, a_hi, a_lo8, n)
                    if n == 0 and m + 1 < MT:
                        a_prev = prep(m + 1)
                    epilogue(n, y_sb, pt, pl)
                    nc.gpsimd.dma_start(
                        y_d[m * 128:(m + 1) * 128, n * 512:(n + 1) * 512],
                        y_sb[:, n * 512:(n + 1) * 512],
                    )

    nc.compile()
    return nc


def kernel(x: np.ndarray, w: np.ndarray) -> np.ndarray:
    import ml_dtypes
    from concourse.bass_utils import run_bass_kernel_spmd

    assert x.shape == (BATCH, N_IN) and w.shape == (N_IN, N_OUT)
    x = np.ascontiguousarray(x, dtype=np.float32)
    w = np.ascontiguousarray(w, dtype=np.float32)

    if "nc" not in _CACHED:
        _CACHED["nc"] = _build_nc()
    nc = _CACHED["nc"]

    # static weight preprocessing: sign(clip(w)), exactly representable
    s8_full = np.sign(np.clip(w, -1.0, 1.0)).astype(ml_dtypes.float8_e4m3)

    in_maps = []
    for c in range(8):
        bg, ng = divmod(c, N_GROUPS)
        # x^T pre-tiled to [m_tile, partition(k%128), k_tile, b] so every
        # DMA is a contiguous per-partition burst
        xt_sh = x[bg * B_SH:(bg + 1) * B_SH, :].T  # [N_IN, B_SH]
        xt_tiled = np.ascontiguousarray(
            xt_sh.reshape(KT, 128, MT, 128).transpose(2, 1, 0, 3)
        )
        # s8 pre-laid as [partition, k_tile, n]
        s8_sh = s8_full[:, ng * N_SH:(ng + 1) * N_SH]
        s8_pre = np.ascontiguousarray(
            s8_sh.reshape(KT, 128, N_SH).transpose(1, 0, 2)
        )
        in_maps.append({"xt": xt_tiled, "s8": s8_pre})

    trace = os.environ.get("PH_KERNEL_TRACE", "") == "1"
    res = run_bass_kernel_spmd(
        nc, in_maps, core_ids=list(range(8)), trace=trace,
        **({"trace_cores": list(range(8))} if trace else {}),
    )
    if trace:
        _CACHED["last_result"] = res

    y = np.empty((BATCH, N_OUT), dtype=np.float32)
    for c, r in enumerate(res.results):
        bg, ng = divmod(c, N_GROUPS)
        y[bg * B_SH:(bg + 1) * B_SH, ng * N_SH:(ng + 1) * N_SH] = r["y"]
    return y


# revision 12
# speedup vs baseline: 1.0606x; 1.0013x over previous
"""AcidBaseDense Trainium2 kernel.

Math (reference, f32):
    bw   = sign(clip(w, -1, 1))                    in {-1, 0, +1}
    h    = 10^(-x);  oh = 1e-14 / h                (oh <= 1e-13 << f32 eps of h)
    r    = (h*0.1) @ bw - (oh*0.1) @ bw            == (h*0.1) @ bw  at f32 precision
    conc = |r| / 409.6
    ph   = -log10(conc)               if r >= 0
         = -log10(1e-14 / conc)       if r <  0

Kernel strategy:
  * host: pre-transpose x so the device loads x^T with n_in on partitions
    (contraction dim must sit on partitions for the PE); binarize the static
    weights (sign(clip(w)) is deterministic weight preprocessing, exact in
    fp16/fp8); shard 2-way over batch x 4-way over n_out across 8 cores.
  * device: A^T = 0.1*10^(-x^T) via one ACT Exp pass.  Precision split:
      A = A_hi (fp16) + A_lo,  |A_lo| <= 2^-11 |A|
    hi pass: fp16 matmul (full rate), S in fp16 (exact +-1/0)
    lo pass: fp8e4 DoubleRow matmul (0.5 cyc/row) on A_lo * 2^18 quantized
      to e4m3 (error 2^-4 * 2^-11 = 2^-15 per term ~ f32-grade), S in fp8.
    Accumulate each pass in its own PSUM bank; combine in the epilogue:
      r = psum_hi + 2^-18 * psum_lo.
  * epilogue in log space avoids any division:
      L = ln(max(|r|, tiny));  u = L*(-1/ln10) + (log10(409.6) - 7)
      ph = 7 + sign(r) * u
"""

import os
import sys

for _p in ("/opt/trn_rl_repo", "/root/.axon_site/_ro/trn_rl_repo"):
    if os.path.isdir(_p) and _p not in sys.path:
        sys.path.insert(0, _p)

import numpy as np

BATCH = 4096
N_IN = 4096
N_OUT = 4096
B_GROUPS = 2           # batch shards
N_GROUPS = 4           # n_out shards
B_SH = BATCH // B_GROUPS      # 2048 batch rows per core
N_SH = N_OUT // N_GROUPS      # 1024 out cols per core
KT = N_IN // 128              # 32 contraction tiles
MT = B_SH // 128              # 16 batch tiles per core
NCHUNK = 2                    # two 512-wide PSUM chunks per batch tile

LN10 = float(np.log(10.0))
U_SCALE = -1.0 / LN10
U_BIAS = float(np.log10(4096 * 0.1) - 7.0)
LO_SCALE = float(2.0 ** 18)
LO_INV = float(2.0 ** -18)

_CACHED = {}


def _build_nc():
    import concourse.bacc as bacc
    import concourse.mybir as mybir
    import concourse.tile as tile

    F32 = mybir.dt.float32
    FP16 = mybir.dt.float16
    FP8 = mybir.dt.float8e4
    BF16 = mybir.dt.bfloat16
    AFT = mybir.ActivationFunctionType

    nc = bacc.Bacc(trn_type="TRN2")
    xt_d = nc.dram_tensor("xt", [MT, 128, KT, 128], F32, kind="ExternalInput")
    s8_d = nc.dram_tensor("s8", [128, KT, N_SH], FP8, kind="ExternalInput")
    y_d = nc.dram_tensor("y", [B_SH, N_SH], F32, kind="ExternalOutput")

    with tile.TileContext(nc) as tc:
        with (
            tc.tile_pool(name="spool", bufs=1) as spool,
            tc.tile_pool(name="mpool", bufs=3) as mpool,
            tc.tile_pool(name="epool", bufs=2) as epool,
            tc.tile_pool(name="cpool", bufs=1) as cpool,
            tc.tile_pool(name="ph_pool", bufs=4, space="PSUM") as ph_pool,
            tc.tile_pool(name="pl_pool", bufs=2, space="PSUM") as pl_pool,
        ):
            bias_ln10 = cpool.tile([128, 1], F32, tag="bias")
            nc.gpsimd.memset(bias_ln10[:], -LN10)

            # m-tile prep: load x^T in 8-ktile quarters (separate tiles for
            # fine-grained deps), exp in place, split hi/lo per quarter
            Q = N_IN // 4

            def prep(m):
                a_hi, a_lo8 = [], []
                for q in range(4):
                    a32q = mpool.tile([128, Q], F32, tag=f"a32q{q}")
                    nc.sync.dma_start(
                        a32q[:].rearrange("p (t b) -> p t b", b=128),
                        xt_d[m, :, q * (KT // 4):(q + 1) * (KT // 4), :],
                    )
                    nc.scalar.activation(
                        a32q[:], a32q[:], AFT.Exp,
                        bias=bias_ln10[:], scale=-LN10,
                    )
                    # a_hi holds A * 2^18 (exact power-of-2 scale) in fp16;
                    # lo8 = e4m3(A*2^18 - a_hi_scaled) in one fused op
                    ahq = mpool.tile([128, Q], FP16, tag=f"ahiq{q}")
                    nc.vector.tensor_scalar_mul(ahq[:], a32q[:], LO_SCALE)
                    al8q = mpool.tile([128, Q], FP8, tag=f"alo8q{q}")
                    nc.vector.scalar_tensor_tensor(
                        al8q[:], a32q[:], LO_SCALE, ahq[:],
                        op0=mybir.AluOpType.mult,
                        op1=mybir.AluOpType.subtract,
                    )
                    a_hi.append(ahq)
                    a_lo8.append(al8q)
                return a_hi, a_lo8

            # S: host-binarized sign(w), exact in fp8 (hi pass reads it as
            # the moving operand of a mixed fp16 x fp8 matmul - bit-exact)
            s8 = spool.tile([128, KT, N_SH], FP8, tag="s8")

            def load_s(q):
                tq = KT // 8
                nc.sync.dma_start(
                    s8[:, q * tq:(q + 1) * tq, :],
                    s8_d[:, q * tq:(q + 1) * tq, :],
                )

            def hilo_chain(m, a_hi, a_lo8, n):
                # interleave hi (fp16) and lo (fp8 DoubleRow) matmuls so the
                # DR LDWEIGHTS (~213ns, FWL off) hides under hi streaming
                pt = ph_pool.tile([128, 512], F32, tag="ph")
                pl = pl_pool.tile([128, 512], F32, tag="pl")
                T = KT // 2
                for t in range(T):
                    for j in (0, 1):
                        kt = 2 * t + j
                        q, kq = divmod(kt, KT // 4)
                        nc.tensor.matmul(
                            pt[:],
                            a_hi[q][:, kq * 128:(kq + 1) * 128],
                            s8[:, kt, n * 512:(n + 1) * 512],
                            start=(kt == 0),
                            stop=(kt == KT - 1),
                            skip_group_check=True,
                        )
                    q, tq = divmod(2 * t, KT // 4)
                    lhs = a_lo8[q][:, tq * 128:(tq + 2) * 128].rearrange(
                        "p (j m) -> p j m", j=2
                    )
                    rhs = s8[:, 2 * t:2 * t + 2, n * 512:(n + 1) * 512]
                    nc.tensor.matmul(
                        pl[:], lhs, rhs,
                        start=(t == 0), stop=(t == T - 1),
                        perf_mode=mybir.MatmulPerfMode.DoubleRow,
                        skip_group_check=True,
                    )
                return pt, pl

            def epilogue(n, y_sb, pt, pl):
                    # r = 2^-18 * (psum_hi + psum_lo), both scaled by 2^18
                    tr = epool.tile([128, 512], F32, tag="tr")
                    nc.scalar.activation(
                        tr[:], pl[:], AFT.Copy, bias=0.0, scale=LO_INV
                    )
                    nc.vector.scalar_tensor_tensor(
                        tr[:], pt[:], LO_INV, tr[:],
                        op0=mybir.AluOpType.mult,
                        op1=mybir.AluOpType.add,
                    )
                    # ph = 7 + sign(r) * (ln|r| * U_SCALE + U_BIAS)
                    tab = epool.tile([128, 512], F32, tag="tab")
                    nc.scalar.activation(tab[:], tr[:], AFT.Abs)
                    nc.vector.tensor_scalar_max(tab[:], tab[:], 1e-30)
                    tln = epool.tile([128, 512], F32, tag="tln")
                    nc.scalar.activation(tln[:], tab[:], AFT.Ln)
                    nc.scalar.activation(
                        tln[:], tln[:], AFT.Copy, bias=U_BIAS, scale=U_SCALE
                    )
                    tsg = epool.tile([128, 512], F32, tag="tsg")
                    nc.scalar.activation(tsg[:], tr[:], AFT.Sign)
                    ych = y_sb[:, n * 512:(n + 1) * 512]
                    nc.vector.tensor_mul(ych, tsg[:], tln[:])
                    nc.vector.tensor_scalar_add(ych, ych, 7.0)

            # ---- pipeline: prep one m-tile ahead
            load_s(0)
            load_s(1)
            a_prev = prep(0)
            for q in range(2, 8):
                load_s(q)
            for m in range(MT):
                a_hi, a_lo8 = a_prev
                y_sb = epool.tile([128, N_SH], F32, tag="y_sb")
                for n in range(NCHUNK):
                    pt, pl = hilo_chain(m, a_hi, a_lo8, n)
                    if n == 0 and m + 1 < MT:
                        a_prev = prep(m + 1)
                    epilogue(n, y_sb, pt, pl)
                    nc.gpsimd.dma_start(
                        y_d[m * 128:(m + 1) * 128, n * 512:(n + 1) * 512],
                        y_sb[:, n * 512:(n + 1) * 512],
                    )

    nc.compile()
    return nc


def kernel(x: np.ndarray, w: np.ndarray) -> np.ndarray:
    import ml_dtypes
    from concourse.bass_utils import run_bass_kernel_spmd

    assert x.shape == (BATCH, N_IN) and w.shape == (N_IN, N_OUT)
    x = np.ascontiguousarray(x, dtype=np.float32)
    w = np.ascontiguousarray(w, dtype=np.float32)

    if "nc" not in _CACHED:
        _CACHED["nc"] = _build_nc()
    nc = _CACHED["nc"]

    # static weight preprocessing: sign(clip(w)), exactly representable
    s8_full = np.sign(np.clip(w, -1.0, 1.0)).astype(ml_dtypes.float8_e4m3)

    in_maps = []
    for c in range(8):
        bg, ng = divmod(c, N_GROUPS)
        # x^T pre-tiled to [m_tile, partition(k%128), k_tile, b] so every
        # DMA is a contiguous per-partition burst
        xt_sh = x[bg * B_SH:(bg + 1) * B_SH, :].T  # [N_IN, B_SH]
        xt_tiled = np.ascontiguousarray(
            xt_sh.reshape(KT, 128, MT, 128).transpose(2, 1, 0, 3)
        )
        # s8 pre-laid as [partition, k_tile, n]
        s8_sh = s8_full[:, ng * N_SH:(ng + 1) * N_SH]
        s8_pre = np.ascontiguousarray(
            s8_sh.reshape(KT, 128, N_SH).transpose(1, 0, 2)
        )
        in_maps.append({"xt": xt_tiled, "s8": s8_pre})

    trace = os.environ.get("PH_KERNEL_TRACE", "") == "1"
    kwargs = {"trace_cores": list(range(8))} if trace else {}
    try:
        res = run_bass_kernel_spmd(
            nc, in_maps, core_ids=list(range(8)), trace=trace, **kwargs
        )
    except Exception as e:  # transient NRT_EXEC_UNIT_UNRECOVERABLE seen rarely
        if "UNRECOVERABLE" not in str(e) and "UNAVAILABLE" not in str(e):
            raise
        import time
        time.sleep(5.0)
        res = run_bass_kernel_spmd(
            nc, in_maps, core_ids=list(range(8)), trace=trace, **kwargs
        )
    if trace:
        _CACHED["last_result"] = res

    y = np.empty((BATCH, N_OUT), dtype=np.float32)
    for c, r in enumerate(res.results):
        bg, ng = divmod(c, N_GROUPS)
        y[bg * B_SH:(bg + 1) * B_SH, ng * N_SH:(ng + 1) * N_SH] = r["y"]
    return y
